# revision 14
# baseline (speedup 1.0000x reference)
"""MoE (Gemma4Experts) Trainium2 kernel.

T=8192 tokens, H=2048 hidden, I=4096 intermediate, E=8 experts, top-2.

Strategy: expert parallelism across the 8 NeuronCores. Host computes the
routing (sort token/k-slot pairs by expert, merging duplicate top-k hits),
gathers each expert's token rows, and ships per-core inputs:
  xt  [16,128,CAP]  bf16  gathered tokens, transposed (K-major for matmul)
  wgu [32,128,4096] bf16  gate+up weights, pre-tiled as lhsT tiles
  wd  [32,128,2048] bf16  down weights, K-major (rhs tiles)
  wgt [128,CAP/128] f32   per-row combine weights
Each core computes y = gelu_tanh(x@Wg^T) * (x@Wu^T) @ Wd^T * w for its rows;
host scatters the weighted rows back into the full [T,H] output (each token
appears in exactly one k0-stream row and at most one k1-stream row, so the
combine is one scatter-assign plus one scatter-add with unique indices).
"""

import sys

sys.path.insert(0, "/opt/trn_rl_repo")

from contextlib import ExitStack

import numpy as np
import ml_dtypes


def _ensure_ntff_hook():
    """bass_utils' trace path imports antenv.axon_hooks, which some images
    lack; provide it (wired to the libaxon ctypes impl when available)."""
    try:
        import antenv.axon_hooks  # noqa: F401
        return
    except ImportError:
        pass
    import types

    mod = types.ModuleType("antenv.axon_hooks")
    state = [None]
    mod.set_axon_ntff_profile_hook = lambda h: state.__setitem__(0, h)
    mod.get_axon_ntff_profile_hook = lambda: state[0]
    sys.modules["antenv.axon_hooks"] = mod
    try:
        import antenv

        antenv.axon_hooks = mod
    except ImportError:
        pass
    try:
        from trn_agent_boot.trn_boot import _ntff_profile_via_ctypes

        state[0] = _ntff_profile_via_ctypes("/opt/axon/libaxon_pjrt.so")
    except Exception:
        pass


_ensure_ntff_hook()

T, H, I_DIM, E = 8192, 2048, 4096, 8
NCORES = 8
BF16 = ml_dtypes.bfloat16

_nc_cache = {}
_w_cache = {}
LAST_RESULT = None


def _build_nc(cap, act_name="Gelu_apprx_tanh"):
    import concourse.bass as bass
    import concourse.tile as tile
    from concourse import bacc, mybir

    f32 = mybir.dt.float32
    bf16 = mybir.dt.bfloat16
    GELU = getattr(mybir.ActivationFunctionType, act_name)

    assert cap % 1024 == 0
    npass = cap // 1024

    nc = bacc.Bacc("TRN2", target_bir_lowering=False, debug=False, num_devices=NCORES)
    xt_d = nc.declare_dram_parameter("xt", [128, 16, cap], bf16, isOutput=False)
    wgu_d = nc.declare_dram_parameter("wgu", [32, 128, 4096], bf16, isOutput=False)
    wd_d = nc.declare_dram_parameter("wd", [128, 32, 2048], bf16, isOutput=False)
    wgt_d = nc.declare_dram_parameter("wgt", [128, cap // 128], f32, isOutput=False)
    y_d = nc.declare_dram_parameter("y", [cap, 2048], bf16, isOutput=True)

    with ExitStack() as ctx:
        tc = ctx.enter_context(tile.TileContext(nc))
        const_pool = ctx.enter_context(tc.tile_pool(name="const", bufs=1))
        x_pool = ctx.enter_context(tc.tile_pool(name="x", bufs=1))
        at_pool = ctx.enter_context(tc.tile_pool(name="at", bufs=1))
        wgu_pool = ctx.enter_context(tc.tile_pool(name="wgu", bufs=5))
        wd_pool = ctx.enter_context(tc.tile_pool(name="wd", bufs=2))
        g_pool = ctx.enter_context(tc.tile_pool(name="g", bufs=2))
        y_pool = ctx.enter_context(tc.tile_pool(name="y", bufs=2))
        psum_pool = ctx.enter_context(tc.tile_pool(name="psum", bufs=2, space="PSUM"))

        wgt_sb = const_pool.tile([128, cap // 128], f32)
        nc.sync.dma_start(wgt_sb[:], wgt_d[:, :])

        for p in range(npass):
            # ---- Phase 1+2: A^T[i, m] = gelu(gate) * up for this pass's 1024 tokens
            # X split per 512-token m-chunk so the first matmul group only
            # waits on half the X load; weights stream on the gpsimd DMA
            # queue so they don't serialize behind X on the sync queue.
            xm = []
            for m in range(2):
                xt_m = x_pool.tile([128, 16, 512], bf16, tag=f"xtile{m}")
                nc.sync.dma_start(
                    xt_m[:], xt_d[:, :, bass.ds(p * 1024 + m * 512, 512)]
                )
                xm.append(xt_m)
            at = at_pool.tile([128, 32, 1024], bf16, tag="at")
            for n in range(32):
                wt = wgu_pool.tile([128, 4096], bf16, tag="wt")
                nc.gpsimd.dma_start(wt[:], wgu_d[n, :, :])
                for m in range(2):
                    ms = bass.ds(m * 512, 512)
                    ps_g = psum_pool.tile([128, 512], f32, tag="psg", bufs=2)
                    ps_u = psum_pool.tile([128, 512], f32, tag="psu", bufs=2)
                    for k in range(16):
                        nc.tensor.matmul(
                            ps_g[:],
                            wt[:, bass.ds(k * 128, 128)],
                            xm[m][:, k],
                            start=(k == 0),
                            stop=(k == 15),
                        )
                    for k in range(16):
                        nc.tensor.matmul(
                            ps_u[:],
                            wt[:, bass.ds((16 + k) * 128, 128)],
                            xm[m][:, k],
                            start=(k == 0),
                            stop=(k == 15),
                        )
                    g = g_pool.tile([128, 512], f32, tag="g")
                    nc.scalar.activation(g[:], ps_g[:], GELU)
                    nc.vector.tensor_mul(at[:, n, ms], g[:], ps_u[:])

            # ---- Phase 3: Y[m, h] = A @ Wd^T, scaled by combine weight
            for h in range(4):
                hs = bass.ds(h * 512, 512)
                wdc = wd_pool.tile([128, 32, 512], bf16, tag="wdc")
                nc.gpsimd.dma_start(wdc[:], wd_d[:, :, hs])
                for t in range(8):
                    ps_y = psum_pool.tile([128, 512], f32, tag="psy", bufs=4)
                    for k in range(32):
                        nc.tensor.matmul(
                            ps_y[:],
                            at[:, k, bass.ds(t * 128, 128)],
                            wdc[:, k],
                            start=(k == 0),
                            stop=(k == 31),
                        )
                    ysb = y_pool.tile([128, 512], bf16, tag="ysb")
                    tg = p * 8 + t
                    nc.scalar.mul(ysb[:], ps_y[:], wgt_sb[:, bass.ds(tg, 1)])
                    nc.sync.dma_start(
                        y_d[bass.ds(p * 1024 + t * 128, 128), hs], ysb[:]
                    )
    nc.compile()
    return nc


def _get_nc(cap):
    if cap not in _nc_cache:
        _nc_cache[cap] = _build_nc(cap)
    return _nc_cache[cap]


def _prep_weights(gate_up_proj, down_proj):
    key = (id(gate_up_proj), id(down_proj))
    if key in _w_cache:
        return _w_cache[key]
    wgu_list, wd_list = [], []
    for e in range(E):
        g = np.asarray(gate_up_proj[e][:I_DIM]).astype(BF16)  # [4096, 2048]
        u = np.asarray(gate_up_proj[e][I_DIM:]).astype(BF16)
        # tile (n,c,k,p) -> (n,p,k,c); lhsT tile [K=128 h, M=128 i]
        gt = np.ascontiguousarray(
            g.reshape(32, 128, 16, 128).transpose(0, 3, 2, 1)
        )
        ut = np.ascontiguousarray(
            u.reshape(32, 128, 16, 128).transpose(0, 3, 2, 1)
        )
        wgu = np.stack([gt, ut], axis=2).reshape(32, 128, 4096)  # (n,p,(half,k,c))
        wgu_list.append(np.ascontiguousarray(wgu))
        d = np.asarray(down_proj[e]).astype(BF16)  # [2048, 4096] (H, I)
        # [p(i within tile), k(i tile), h] so each h-chunk is one DMA
        wd = np.ascontiguousarray(d.T.reshape(32, 128, 2048).transpose(1, 0, 2))
        wd_list.append(wd)
    val = (wgu_list, wd_list, gate_up_proj, down_proj)  # keep refs alive
    _w_cache[key] = val
    return val


def kernel(hidden_states, top_k_index, top_k_weights, gate_up_proj, down_proj):
    global LAST_RESULT
    from concourse.bass_utils import run_bass_kernel_spmd

    hidden = np.asarray(hidden_states, dtype=np.float32)
    idx = np.asarray(top_k_index).astype(np.int64)
    wts = np.asarray(top_k_weights, dtype=np.float32)

    # ---- routing on host (merge duplicate top-k hits of the same expert)
    dup = idx[:, 0] == idx[:, 1]
    w0 = np.where(dup, wts[:, 0] + wts[:, 1], wts[:, 0])
    e0 = idx[:, 0]
    tok1 = np.nonzero(~dup)[0]
    e1 = idx[tok1, 1]
    w1 = wts[tok1, 1]

    routes = []
    maxn = 0
    for e in range(E):
        t0 = np.nonzero(e0 == e)[0]
        t1 = tok1[e1 == e]
        routes.append((t0, w0[t0], t1, w1[e1 == e]))
        maxn = max(maxn, len(t0) + len(t1))
    cap = max(2048, -(-maxn // 1024) * 1024)

    wgu_list, wd_list, _, _ = _prep_weights(gate_up_proj, down_proj)
    hidden_bf = hidden.astype(BF16)

    in_maps = []
    for e in range(E):
        t0, we0, t1, we1 = routes[e]
        n = len(t0) + len(t1)
        toks = np.concatenate([t0, t1])
        xt = np.zeros((128, 16, cap), dtype=BF16)
        xt[:, :, :n] = hidden_bf[toks].T.reshape(16, 128, n).transpose(1, 0, 2)
        wv = np.zeros(cap, dtype=np.float32)
        wv[: len(t0)] = we0
        wv[len(t0) : n] = we1
        wgt = np.ascontiguousarray(wv.reshape(cap // 128, 128).T)
        in_maps.append(
            {"xt": xt, "wgu": wgu_list[e], "wd": wd_list[e], "wgt": wgt}
        )

    nc = _get_nc(cap)
    res = run_bass_kernel_spmd(nc, in_maps, core_ids=list(range(NCORES)))
    LAST_RESULT = res

    out = np.empty((T, H), dtype=np.float32)
    for e in range(E):
        t0, _, t1, _ = routes[e]
        y = res.results[e]["y"]
        out[t0] = y[: len(t0)].astype(np.float32)
    for e in range(E):
        t0, _, t1, _ = routes[e]
        y = res.results[e]["y"]
        out[t1] += y[len(t0) : len(t0) + len(t1)].astype(np.float32)
    return out


# revision 15
# speedup vs baseline: 1.0132x; 1.0132x over previous
"""MoE (Gemma4Experts) Trainium2 kernel.

T=8192 tokens, H=2048 hidden, I=4096 intermediate, E=8 experts, top-2.

Strategy: expert parallelism across the 8 NeuronCores. Host computes the
routing (sort token/k-slot pairs by expert, merging duplicate top-k hits),
gathers each expert's token rows, and ships per-core inputs:
  xt  [16,128,CAP]  bf16  gathered tokens, transposed (K-major for matmul)
  wgu [32,128,4096] bf16  gate+up weights, pre-tiled as lhsT tiles
  wd  [32,128,2048] bf16  down weights, K-major (rhs tiles)
  wgt [128,CAP/128] f32   per-row combine weights
Each core computes y = gelu_tanh(x@Wg^T) * (x@Wu^T) @ Wd^T * w for its rows;
host scatters the weighted rows back into the full [T,H] output (each token
appears in exactly one k0-stream row and at most one k1-stream row, so the
combine is one scatter-assign plus one scatter-add with unique indices).
"""

import sys

sys.path.insert(0, "/opt/trn_rl_repo")

from contextlib import ExitStack

import numpy as np
import ml_dtypes


def _ensure_ntff_hook():
    """bass_utils' trace path imports antenv.axon_hooks, which some images
    lack; provide it (wired to the libaxon ctypes impl when available)."""
    try:
        import antenv.axon_hooks  # noqa: F401
        return
    except ImportError:
        pass
    import types

    mod = types.ModuleType("antenv.axon_hooks")
    state = [None]
    mod.set_axon_ntff_profile_hook = lambda h: state.__setitem__(0, h)
    mod.get_axon_ntff_profile_hook = lambda: state[0]
    sys.modules["antenv.axon_hooks"] = mod
    try:
        import antenv

        antenv.axon_hooks = mod
    except ImportError:
        pass
    try:
        from trn_agent_boot.trn_boot import _ntff_profile_via_ctypes

        state[0] = _ntff_profile_via_ctypes("/opt/axon/libaxon_pjrt.so")
    except Exception:
        pass


_ensure_ntff_hook()

T, H, I_DIM, E = 8192, 2048, 4096, 8
NCORES = 8
BF16 = ml_dtypes.bfloat16

_nc_cache = {}
_w_cache = {}
LAST_RESULT = None


def _build_nc(cap, act_name="Gelu_apprx_tanh"):
    import concourse.bass as bass
    import concourse.tile as tile
    from concourse import bacc, mybir

    f32 = mybir.dt.float32
    bf16 = mybir.dt.bfloat16
    GELU = getattr(mybir.ActivationFunctionType, act_name)

    assert cap % 1024 == 0
    npass = cap // 1024

    nc = bacc.Bacc("TRN2", target_bir_lowering=False, debug=False, num_devices=NCORES)
    xt_d = nc.declare_dram_parameter("xt", [cap // 512, 128, 16, 512], bf16, isOutput=False)
    wgu_d = nc.declare_dram_parameter("wgu", [32, 128, 4096], bf16, isOutput=False)
    wd_d = nc.declare_dram_parameter("wd", [4, 128, 32, 512], bf16, isOutput=False)
    wgt_d = nc.declare_dram_parameter("wgt", [128, cap // 128], f32, isOutput=False)
    y_d = nc.declare_dram_parameter("y", [cap, 2048], bf16, isOutput=True)

    with ExitStack() as ctx:
        tc = ctx.enter_context(tile.TileContext(nc))
        const_pool = ctx.enter_context(tc.tile_pool(name="const", bufs=1))
        x_pool = ctx.enter_context(tc.tile_pool(name="x", bufs=1))
        at_pool = ctx.enter_context(tc.tile_pool(name="at", bufs=1))
        wgu_pool = ctx.enter_context(tc.tile_pool(name="wgu", bufs=5))
        wd_pool = ctx.enter_context(tc.tile_pool(name="wd", bufs=2))
        g_pool = ctx.enter_context(tc.tile_pool(name="g", bufs=2))
        y_pool = ctx.enter_context(tc.tile_pool(name="y", bufs=2))
        psum_pool = ctx.enter_context(tc.tile_pool(name="psum", bufs=2, space="PSUM"))

        wgt_sb = const_pool.tile([128, cap // 128], f32)
        nc.sync.dma_start(wgt_sb[:], wgt_d[:, :])

        for p in range(npass):
            # ---- Phase 1+2: A^T[i, m] = gelu(gate) * up for this pass's 1024 tokens
            # X split per 512-token m-chunk so the first matmul group only
            # waits on half the X load; weights stream on the gpsimd DMA
            # queue so they don't serialize behind X on the sync queue.
            xm = []
            for m in range(2):
                xt_m = x_pool.tile([128, 16, 512], bf16, tag=f"xtile{m}")
                nc.sync.dma_start(xt_m[:], xt_d[p * 2 + m])
                xm.append(xt_m)
            at = at_pool.tile([128, 32, 1024], bf16, tag="at")
            for n in range(32):
                wt = wgu_pool.tile([128, 4096], bf16, tag="wt")
                nc.gpsimd.dma_start(wt[:], wgu_d[n, :, :])
                for m in range(2):
                    ms = bass.ds(m * 512, 512)
                    ps_g = psum_pool.tile([128, 512], f32, tag="psg", bufs=2)
                    ps_u = psum_pool.tile([128, 512], f32, tag="psu", bufs=2)
                    for k in range(16):
                        nc.tensor.matmul(
                            ps_g[:],
                            wt[:, bass.ds(k * 128, 128)],
                            xm[m][:, k],
                            start=(k == 0),
                            stop=(k == 15),
                        )
                    for k in range(16):
                        nc.tensor.matmul(
                            ps_u[:],
                            wt[:, bass.ds((16 + k) * 128, 128)],
                            xm[m][:, k],
                            start=(k == 0),
                            stop=(k == 15),
                        )
                    g = g_pool.tile([128, 512], f32, tag="g")
                    nc.scalar.activation(g[:], ps_g[:], GELU)
                    nc.vector.tensor_mul(at[:, n, ms], g[:], ps_u[:])

            # ---- Phase 3: Y[m, h] = A @ Wd^T, scaled by combine weight
            for h in range(4):
                hs = bass.ds(h * 512, 512)
                wdc = wd_pool.tile([128, 32, 512], bf16, tag="wdc")
                nc.gpsimd.dma_start(wdc[:], wd_d[h])
                for t in range(8):
                    ps_y = psum_pool.tile([128, 512], f32, tag="psy", bufs=4)
                    for k in range(32):
                        nc.tensor.matmul(
                            ps_y[:],
                            at[:, k, bass.ds(t * 128, 128)],
                            wdc[:, k],
                            start=(k == 0),
                            stop=(k == 31),
                        )
                    ysb = y_pool.tile([128, 512], bf16, tag="ysb")
                    tg = p * 8 + t
                    nc.scalar.mul(ysb[:], ps_y[:], wgt_sb[:, bass.ds(tg, 1)])
                    nc.sync.dma_start(
                        y_d[bass.ds(p * 1024 + t * 128, 128), hs], ysb[:]
                    )
    nc.compile()
    return nc


def _get_nc(cap):
    if cap not in _nc_cache:
        _nc_cache[cap] = _build_nc(cap)
    return _nc_cache[cap]


def _prep_weights(gate_up_proj, down_proj):
    key = (id(gate_up_proj), id(down_proj))
    if key in _w_cache:
        return _w_cache[key]
    wgu_list, wd_list = [], []
    for e in range(E):
        g = np.asarray(gate_up_proj[e][:I_DIM]).astype(BF16)  # [4096, 2048]
        u = np.asarray(gate_up_proj[e][I_DIM:]).astype(BF16)
        # tile (n,c,k,p) -> (n,p,k,c); lhsT tile [K=128 h, M=128 i]
        gt = np.ascontiguousarray(
            g.reshape(32, 128, 16, 128).transpose(0, 3, 2, 1)
        )
        ut = np.ascontiguousarray(
            u.reshape(32, 128, 16, 128).transpose(0, 3, 2, 1)
        )
        wgu = np.stack([gt, ut], axis=2).reshape(32, 128, 4096)  # (n,p,(half,k,c))
        wgu_list.append(np.ascontiguousarray(wgu))
        d = np.asarray(down_proj[e]).astype(BF16)  # [2048, 4096] (H, I)
        # chunk-major [h-chunk, p(i in tile), k(i tile), h in chunk]:
        # each 512-wide h-chunk is one fully contiguous DMA
        wd = np.ascontiguousarray(
            d.T.reshape(32, 128, 4, 512).transpose(2, 1, 0, 3)
        )
        wd_list.append(wd)
    val = (wgu_list, wd_list, gate_up_proj, down_proj)  # keep refs alive
    _w_cache[key] = val
    return val


def kernel(hidden_states, top_k_index, top_k_weights, gate_up_proj, down_proj):
    global LAST_RESULT
    from concourse.bass_utils import run_bass_kernel_spmd

    hidden = np.asarray(hidden_states, dtype=np.float32)
    idx = np.asarray(top_k_index).astype(np.int64)
    wts = np.asarray(top_k_weights, dtype=np.float32)

    # ---- routing on host (merge duplicate top-k hits of the same expert)
    dup = idx[:, 0] == idx[:, 1]
    w0 = np.where(dup, wts[:, 0] + wts[:, 1], wts[:, 0])
    e0 = idx[:, 0]
    tok1 = np.nonzero(~dup)[0]
    e1 = idx[tok1, 1]
    w1 = wts[tok1, 1]

    routes = []
    maxn = 0
    for e in range(E):
        t0 = np.nonzero(e0 == e)[0]
        t1 = tok1[e1 == e]
        routes.append((t0, w0[t0], t1, w1[e1 == e]))
        maxn = max(maxn, len(t0) + len(t1))
    cap = max(2048, -(-maxn // 1024) * 1024)

    wgu_list, wd_list, _, _ = _prep_weights(gate_up_proj, down_proj)
    hidden_bf = hidden.astype(BF16)

    in_maps = []
    for e in range(E):
        t0, we0, t1, we1 = routes[e]
        n = len(t0) + len(t1)
        toks = np.concatenate([t0, t1])
        xt = np.zeros((16, 128, cap), dtype=BF16)
        xt[:, :, :n] = hidden_bf[toks].T.reshape(16, 128, n)
        # chunk-major [m-chunk, p(h in tile), k(h tile), m in chunk]
        xt = np.ascontiguousarray(
            xt.reshape(16, 128, cap // 512, 512).transpose(2, 1, 0, 3)
        )
        wv = np.zeros(cap, dtype=np.float32)
        wv[: len(t0)] = we0
        wv[len(t0) : n] = we1
        wgt = np.ascontiguousarray(wv.reshape(cap // 128, 128).T)
        in_maps.append(
            {"xt": xt, "wgu": wgu_list[e], "wd": wd_list[e], "wgt": wgt}
        )

    nc = _get_nc(cap)
    res = run_bass_kernel_spmd(nc, in_maps, core_ids=list(range(NCORES)))
    LAST_RESULT = res

    out = np.empty((T, H), dtype=np.float32)
    for e in range(E):
        t0, _, t1, _ = routes[e]
        y = res.results[e]["y"]
        out[t0] = y[: len(t0)].astype(np.float32)
    for e in range(E):
        t0, _, t1, _ = routes[e]
        y = res.results[e]["y"]
        out[t1] += y[len(t0) : len(t0) + len(t1)].astype(np.float32)
    return out


# revision 16
# speedup vs baseline: 1.0137x; 1.0005x over previous
"""MoE (Gemma4Experts) Trainium2 kernel.

T=8192 tokens, H=2048 hidden, I=4096 intermediate, E=8 experts, top-2.

Strategy: expert parallelism across the 8 NeuronCores. Host computes the
routing (sort token/k-slot pairs by expert, merging duplicate top-k hits),
gathers each expert's token rows, and ships per-core inputs:
  xt  [16,128,CAP]  bf16  gathered tokens, transposed (K-major for matmul)
  wgu [32,128,4096] bf16  gate+up weights, pre-tiled as lhsT tiles
  wd  [32,128,2048] bf16  down weights, K-major (rhs tiles)
  wgt [128,CAP/128] f32   per-row combine weights
Each core computes y = gelu_tanh(x@Wg^T) * (x@Wu^T) @ Wd^T * w for its rows;
host scatters the weighted rows back into the full [T,H] output (each token
appears in exactly one k0-stream row and at most one k1-stream row, so the
combine is one scatter-assign plus one scatter-add with unique indices).
"""

import sys

sys.path.insert(0, "/opt/trn_rl_repo")

from contextlib import ExitStack

import numpy as np
import ml_dtypes


def _ensure_ntff_hook():
    """bass_utils' trace path imports antenv.axon_hooks, which some images
    lack; provide it (wired to the libaxon ctypes impl when available)."""
    try:
        import antenv.axon_hooks  # noqa: F401
        return
    except ImportError:
        pass
    import types

    mod = types.ModuleType("antenv.axon_hooks")
    state = [None]
    mod.set_axon_ntff_profile_hook = lambda h: state.__setitem__(0, h)
    mod.get_axon_ntff_profile_hook = lambda: state[0]
    sys.modules["antenv.axon_hooks"] = mod
    try:
        import antenv

        antenv.axon_hooks = mod
    except ImportError:
        pass
    try:
        from trn_agent_boot.trn_boot import _ntff_profile_via_ctypes

        state[0] = _ntff_profile_via_ctypes("/opt/axon/libaxon_pjrt.so")
    except Exception:
        pass


_ensure_ntff_hook()

T, H, I_DIM, E = 8192, 2048, 4096, 8
NCORES = 8
BF16 = ml_dtypes.bfloat16

_nc_cache = {}
_w_cache = {}
LAST_RESULT = None


def _build_nc(cap, act_name="Gelu_apprx_tanh"):
    import concourse.bass as bass
    import concourse.tile as tile
    from concourse import bacc, mybir

    f32 = mybir.dt.float32
    bf16 = mybir.dt.bfloat16
    GELU = getattr(mybir.ActivationFunctionType, act_name)

    assert cap % 1024 == 0
    npass = cap // 1024

    nc = bacc.Bacc("TRN2", target_bir_lowering=False, debug=False, num_devices=NCORES)
    xt_d = nc.declare_dram_parameter("xt", [cap // 512, 128, 16, 512], bf16, isOutput=False)
    wgu_d = nc.declare_dram_parameter("wgu", [32, 128, 4096], bf16, isOutput=False)
    wd_d = nc.declare_dram_parameter("wd", [4, 128, 32, 512], bf16, isOutput=False)
    wgt_d = nc.declare_dram_parameter("wgt", [128, cap // 128], f32, isOutput=False)
    y_d = nc.declare_dram_parameter("y", [cap, 2048], bf16, isOutput=True)

    with ExitStack() as ctx:
        tc = ctx.enter_context(tile.TileContext(nc))
        const_pool = ctx.enter_context(tc.tile_pool(name="const", bufs=1))
        x_pool = ctx.enter_context(tc.tile_pool(name="x", bufs=1))
        at_pool = ctx.enter_context(tc.tile_pool(name="at", bufs=1))
        wgu_pool = ctx.enter_context(tc.tile_pool(name="wgu", bufs=5))
        wd_pool = ctx.enter_context(tc.tile_pool(name="wd", bufs=2))
        g_pool = ctx.enter_context(tc.tile_pool(name="g", bufs=2))
        y_pool = ctx.enter_context(tc.tile_pool(name="y", bufs=2))
        psum_pool = ctx.enter_context(tc.tile_pool(name="psum", bufs=2, space="PSUM"))

        wgt_sb = const_pool.tile([128, cap // 128], f32)
        nc.sync.dma_start(wgt_sb[:], wgt_d[:, :])

        for p in range(npass):
            # ---- Phase 1+2: A^T[i, m] = gelu(gate) * up for this pass's 1024 tokens
            # X split per 512-token m-chunk so the first matmul group only
            # waits on half the X load; weights stream on the gpsimd DMA
            # queue so they don't serialize behind X on the sync queue.
            xm = []
            for m in range(2):
                xt_m = x_pool.tile([128, 16, 512], bf16, tag=f"xtile{m}")
                # split across both HW-DGE rings (SP + ACT) for bandwidth
                nc.sync.dma_start(xt_m[:, :8], xt_d[p * 2 + m, :, :8])
                nc.scalar.dma_start(xt_m[:, 8:], xt_d[p * 2 + m, :, 8:])
                xm.append(xt_m)
            at = at_pool.tile([128, 32, 1024], bf16, tag="at")
            for n in range(32):
                wt = wgu_pool.tile([128, 4096], bf16, tag="wt")
                nc.scalar.dma_start(wt[:], wgu_d[n, :, :])
                for m in range(2):
                    ms = bass.ds(m * 512, 512)
                    ps_g = psum_pool.tile([128, 512], f32, tag="psg", bufs=2)
                    ps_u = psum_pool.tile([128, 512], f32, tag="psu", bufs=2)
                    for k in range(16):
                        nc.tensor.matmul(
                            ps_g[:],
                            wt[:, bass.ds(k * 128, 128)],
                            xm[m][:, k],
                            start=(k == 0),
                            stop=(k == 15),
                        )
                    for k in range(16):
                        nc.tensor.matmul(
                            ps_u[:],
                            wt[:, bass.ds((16 + k) * 128, 128)],
                            xm[m][:, k],
                            start=(k == 0),
                            stop=(k == 15),
                        )
                    g = g_pool.tile([128, 512], f32, tag="g")
                    nc.scalar.activation(g[:], ps_g[:], GELU)
                    nc.vector.tensor_mul(at[:, n, ms], g[:], ps_u[:])

            # ---- Phase 3: Y[m, h] = A @ Wd^T, scaled by combine weight
            for h in range(4):
                hs = bass.ds(h * 512, 512)
                wdc = wd_pool.tile([128, 32, 512], bf16, tag="wdc")
                nc.sync.dma_start(wdc[:], wd_d[h])
                for t in range(8):
                    ps_y = psum_pool.tile([128, 512], f32, tag="psy", bufs=4)
                    for k in range(32):
                        nc.tensor.matmul(
                            ps_y[:],
                            at[:, k, bass.ds(t * 128, 128)],
                            wdc[:, k],
                            start=(k == 0),
                            stop=(k == 31),
                        )
                    ysb = y_pool.tile([128, 512], bf16, tag="ysb")
                    tg = p * 8 + t
                    nc.scalar.mul(ysb[:], ps_y[:], wgt_sb[:, bass.ds(tg, 1)])
                    nc.sync.dma_start(
                        y_d[bass.ds(p * 1024 + t * 128, 128), hs], ysb[:]
                    )
    nc.compile()
    return nc


def _get_nc(cap):
    if cap not in _nc_cache:
        _nc_cache[cap] = _build_nc(cap)
    return _nc_cache[cap]


def _prep_weights(gate_up_proj, down_proj):
    key = (id(gate_up_proj), id(down_proj))
    if key in _w_cache:
        return _w_cache[key]
    wgu_list, wd_list = [], []
    for e in range(E):
        g = np.asarray(gate_up_proj[e][:I_DIM]).astype(BF16)  # [4096, 2048]
        u = np.asarray(gate_up_proj[e][I_DIM:]).astype(BF16)
        # tile (n,c,k,p) -> (n,p,k,c); lhsT tile [K=128 h, M=128 i]
        gt = np.ascontiguousarray(
            g.reshape(32, 128, 16, 128).transpose(0, 3, 2, 1)
        )
        ut = np.ascontiguousarray(
            u.reshape(32, 128, 16, 128).transpose(0, 3, 2, 1)
        )
        wgu = np.stack([gt, ut], axis=2).reshape(32, 128, 4096)  # (n,p,(half,k,c))
        wgu_list.append(np.ascontiguousarray(wgu))
        d = np.asarray(down_proj[e]).astype(BF16)  # [2048, 4096] (H, I)
        # chunk-major [h-chunk, p(i in tile), k(i tile), h in chunk]:
        # each 512-wide h-chunk is one fully contiguous DMA
        wd = np.ascontiguousarray(
            d.T.reshape(32, 128, 4, 512).transpose(2, 1, 0, 3)
        )
        wd_list.append(wd)
    val = (wgu_list, wd_list, gate_up_proj, down_proj)  # keep refs alive
    _w_cache[key] = val
    return val


def kernel(hidden_states, top_k_index, top_k_weights, gate_up_proj, down_proj):
    global LAST_RESULT
    from concourse.bass_utils import run_bass_kernel_spmd

    hidden = np.asarray(hidden_states, dtype=np.float32)
    idx = np.asarray(top_k_index).astype(np.int64)
    wts = np.asarray(top_k_weights, dtype=np.float32)

    # ---- routing on host (merge duplicate top-k hits of the same expert)
    dup = idx[:, 0] == idx[:, 1]
    w0 = np.where(dup, wts[:, 0] + wts[:, 1], wts[:, 0])
    e0 = idx[:, 0]
    tok1 = np.nonzero(~dup)[0]
    e1 = idx[tok1, 1]
    w1 = wts[tok1, 1]

    routes = []
    maxn = 0
    for e in range(E):
        t0 = np.nonzero(e0 == e)[0]
        t1 = tok1[e1 == e]
        routes.append((t0, w0[t0], t1, w1[e1 == e]))
        maxn = max(maxn, len(t0) + len(t1))
    cap = max(2048, -(-maxn // 1024) * 1024)

    wgu_list, wd_list, _, _ = _prep_weights(gate_up_proj, down_proj)
    hidden_bf = hidden.astype(BF16)

    in_maps = []
    for e in range(E):
        t0, we0, t1, we1 = routes[e]
        n = len(t0) + len(t1)
        toks = np.concatenate([t0, t1])
        xt = np.zeros((16, 128, cap), dtype=BF16)
        xt[:, :, :n] = hidden_bf[toks].T.reshape(16, 128, n)
        # chunk-major [m-chunk, p(h in tile), k(h tile), m in chunk]
        xt = np.ascontiguousarray(
            xt.reshape(16, 128, cap // 512, 512).transpose(2, 1, 0, 3)
        )
        wv = np.zeros(cap, dtype=np.float32)
        wv[: len(t0)] = we0
        wv[len(t0) : n] = we1
        wgt = np.ascontiguousarray(wv.reshape(cap // 128, 128).T)
        in_maps.append(
            {"xt": xt, "wgu": wgu_list[e], "wd": wd_list[e], "wgt": wgt}
        )

    nc = _get_nc(cap)
    res = run_bass_kernel_spmd(nc, in_maps, core_ids=list(range(NCORES)))
    LAST_RESULT = res

    out = np.empty((T, H), dtype=np.float32)
    for e in range(E):
        t0, _, t1, _ = routes[e]
        y = res.results[e]["y"]
        out[t0] = y[: len(t0)].astype(np.float32)
    for e in range(E):
        t0, _, t1, _ = routes[e]
        y = res.results[e]["y"]
        out[t1] += y[len(t0) : len(t0) + len(t1)].astype(np.float32)
    return out
